# revision 10
# baseline (speedup 1.0000x reference)
"""Bass/Trainium2 kernel for nn_AttentionLayer (B=4, S=2048, H=16, DH=64).

Sharding: 8 cores = 4 batches x 2 head-groups (8 heads each). Each core
computes its batch's full S x S attention for its 8 heads; no cross-core
communication. Host slices inputs per core and transposes/concats outputs.

Per-core dataflow (all fp32 storage; matmuls run as float32r bitcasts):
  x_to -> PE-transpose -> toT [m, t] -> K^T [d, t] and V [t, d|1]
  x_from -> PE-transpose -> fromT [m, f] -> Q^T [d, f]
  per head h, per f-half:
    scores^T[t, f] = K_h Q_h^T       (PE, K=64 row-group per head parity)
    probs^T = exp(0.125*scores^T + mask_bias)   (scalar engine, PSUM->SBUF)
    out^T[d|sum, f] += [V_h|1]^T probs^T        (PE, M=65, PSUM-accumulated)
    out = out^T[0:64] * (1/out^T[64])           (DVE + gpsimd broadcast)
Output per core: outT [512, 2048] (head-major rows); host takes outT.T.
"""

import sys

sys.path.insert(0, "/opt/trn_rl_repo")

import numpy as np

import concourse.bass as bass
import concourse.tile as tile
from concourse import bacc, mybir
from concourse.bass_utils import run_bass_kernel_spmd
from concourse.masks import make_identity

B, S, H, DH = 4, 2048, 16, 64
DM = H * DH          # 1024 model dim
HL = 8               # heads per core
DL = HL * DH         # 512 projected dim per core
P = 128
NFT = S // P         # 16 sequence tiles
NMT = DM // P        # 8 model-dim tiles
NDT = DL // P        # 4 projected-dim tiles
FC = 512             # matmul free-dim chunk
VW = DH + 1          # V columns per head incl. ones column

f32 = mybir.dt.float32
f32r = mybir.dt.float32r
AF = mybir.ActivationFunctionType


def _r(ap):
    return ap if ap.dtype == f32r else ap.bitcast(f32r)


def _build_program(s=S):
    nc = bacc.Bacc("TRN2", target_bir_lowering=False, num_devices=8)
    nft = s // P

    x_from_d = nc.dram_tensor("x_from", [s, DM], f32r, kind="ExternalInput")
    x_to_d = nc.dram_tensor("x_to", [s, DM], f32r, kind="ExternalInput")
    wq_d = nc.dram_tensor("wq", [DM, DL], f32r, kind="ExternalInput")
    wk_d = nc.dram_tensor("wk", [DM, DL], f32r, kind="ExternalInput")
    wv_d = nc.dram_tensor("wv", [DM, DL], f32r, kind="ExternalInput")
    bq_d = nc.dram_tensor("bq", [1, DL], f32r, kind="ExternalInput")
    bk_d = nc.dram_tensor("bk", [1, DL], f32r, kind="ExternalInput")
    bv_d = nc.dram_tensor("bv", [1, DL], f32r, kind="ExternalInput")
    mb_d = nc.dram_tensor("mask_bias", [P, nft], f32, kind="ExternalInput")
    outT_d = nc.dram_tensor("outT", [DL, s], f32, kind="ExternalOutput")

    xf_t = x_from_d.rearrange("(ft p) m -> ft p m", p=P)
    xt_t = x_to_d.rearrange("(ft p) m -> ft p m", p=P)

    with tile.TileContext(nc) as tc:
        with tc.tile_pool(name="const", bufs=1) as const, \
             tc.tile_pool(name="big", bufs=1) as big:
            ident_f = const.tile([P, P], f32)
            make_identity(nc, ident_f[:])
            ident = const.tile([P, P], f32r)
            nc.vector.tensor_copy(ident[:], ident_f[:])
            ones_f = const.tile([P, FC], f32)
            nc.gpsimd.memset(ones_f[:], 1.0)
            ones_row = const.tile([1, FC], f32r)
            nc.vector.tensor_copy(ones_row[:], ones_f[0:1, :])
            mb = const.tile([P, nft], f32)
            nc.sync.dma_start(mb[:], mb_d[:])
            bq_sb = const.tile([1, DL], f32r)
            nc.sync.dma_start(bq_sb[:], bq_d[:])
            bk_sb = const.tile([1, DL], f32r)
            nc.sync.dma_start(bk_sb[:], bk_d[:])
            bv_sb = const.tile([1, DL], f32r)
            nc.sync.dma_start(bv_sb[:], bv_d[:])

            QT = big.tile([P, NDT, s], f32r)   # Q^T: [d%128, d//128, f]
            KT = big.tile([P, NDT, s], f32r)   # K^T: [d%128, d//128, t]
            V = big.tile([P, nft, HL * VW], f32r)  # [t%128, t//128, h*65+j]
            nc.vector.tensor_copy(
                V.rearrange("p t (h d) -> p t h d", d=VW)[:, :, :, DH],
                ones_f[:, 0:nft * HL].rearrange("p (t h) -> p t h", h=HL),
            )

            def transpose_in(x_tiled, dst, xpool, tps):
                # dst[m%128, m//128, s] = x[s, m]
                for ft in range(nft):
                    xt = xpool.tile([P, DM], f32r, tag="x")
                    nc.sync.dma_start(xt[:], x_tiled[ft])
                    ps = tps.tile([P, DM], f32r, tag="tp")
                    for mt in range(NMT):
                        sl = slice(mt * P, (mt + 1) * P)
                        nc.tensor.transpose(
                            _r(ps[:, sl]), _r(xt[:, sl]), _r(ident[:])
                        )
                    nc.vector.tensor_copy(
                        dst[:, :, ft * P:(ft + 1) * P],
                        ps.rearrange("p (mt c) -> p mt c", mt=NMT),
                    )

            def project_T(w_dram, b_sb, xT, dst, wpool, pps):
                # dst[d%128, d//128, s] = sum_m w[m, d] * xT[m, s] + b[d]
                for dt in range(NDT):
                    wt = wpool.tile([P, NMT, P], f32r, tag="w")
                    nc.sync.dma_start(
                        wt[:],
                        w_dram[:, dt * P:(dt + 1) * P].rearrange(
                            "(mt p) d -> p mt d", p=P
                        ),
                    )
                    for c in range(s // FC):
                        ps = pps.tile([P, FC], f32, tag="pj")
                        for mt in range(NMT):
                            nc.tensor.matmul(
                                ps[:],
                                lhsT=_r(wt[:, mt, :]),
                                rhs=_r(xT[:, mt, c * FC:(c + 1) * FC]),
                                start=(mt == 0),
                                stop=False,
                            )
                        nc.tensor.matmul(
                            ps[:],
                            lhsT=_r(b_sb[0:1, dt * P:(dt + 1) * P]),
                            rhs=_r(ones_row[0:1, :]),
                            start=False,
                            stop=True,
                        )
                        nc.vector.tensor_copy(
                            dst[:, dt, c * FC:(c + 1) * FC], ps[:]
                        )

            with tc.tile_pool(name="xload", bufs=2) as xpool, \
                 tc.tile_pool(name="wpool", bufs=2) as wpool, \
                 tc.tile_pool(name="tp_ps", bufs=2, space="PSUM") as tps, \
                 tc.tile_pool(name="pj_ps", bufs=2, space="PSUM") as pps:

                with tc.tile_pool(name="toT_pool", bufs=1) as toT_pool:
                    toT = toT_pool.tile([P, NMT, s], f32r)
                    transpose_in(xt_t, toT, xpool, tps)
                    project_T(wk_d, bk_sb, toT, KT, wpool, pps)
                    # V[t, d] = sum_m toT[m, t] * wv[m, d] + bv[d]
                    wv_sb = toT_pool.tile([P, NMT, DL], f32r)
                    nc.sync.dma_start(
                        wv_sb[:], wv_d.rearrange("(mt p) d -> p mt d", p=P)
                    )
                    for tt in range(nft):
                        ps = pps.tile([P, FC], f32, tag="pj")
                        for mt in range(NMT):
                            nc.tensor.matmul(
                                ps[:],
                                lhsT=_r(toT[:, mt, tt * P:(tt + 1) * P]),
                                rhs=_r(wv_sb[:, mt, :]),
                                start=(mt == 0),
                                stop=False,
                            )
                        nc.tensor.matmul(
                            ps[:],
                            lhsT=_r(ones_row[0:1, 0:P]),
                            rhs=_r(bv_sb[:]),
                            start=False,
                            stop=True,
                        )
                        nc.vector.tensor_copy(
                            V.rearrange("p t (h d) -> p t h d", d=VW)[
                                :, tt, :, 0:DH
                            ],
                            ps.rearrange("p (h d) -> p h d", d=DH),
                        )

                with tc.tile_pool(name="fromT_pool", bufs=1) as fromT_pool:
                    fromT = fromT_pool.tile([P, NMT, s], f32r)
                    transpose_in(xf_t, fromT, xpool, tps)
                    project_T(wq_d, bq_sb, fromT, QT, wpool, pps)

            # ---- attention ----
            with tc.tile_pool(name="sc_ps", bufs=2, space="PSUM") as scps, \
                 tc.tile_pool(name="av_ps", bufs=2, space="PSUM") as avps, \
                 tc.tile_pool(name="probs", bufs=3) as prpool, \
                 tc.tile_pool(name="norm", bufs=2) as nrm, \
                 tc.tile_pool(name="outp", bufs=2) as outp:
                FH = min(s, 2 * FC)  # f-half width
                nfh = s // FH
                nck = FH // FC
                for h in range(HL):
                    pr, ro = h // 2, (h % 2) * 64
                    for fh in range(nfh):
                        av = [
                            avps.tile([VW, FC], f32, tag=f"av{i}",
                                      name=f"av{i}")
                            for i in range(nck)
                        ]
                        for tt in range(nft):
                            sc = scps.tile([P, FH], f32, tag="sc")
                            for c2 in range(nck):
                                nc.tensor.matmul(
                                    sc[:, c2 * FC:(c2 + 1) * FC],
                                    lhsT=_r(KT[ro:ro + 64, pr, tt * P:(tt + 1) * P]),
                                    rhs=_r(QT[ro:ro + 64, pr,
                                              fh * FH + c2 * FC:
                                              fh * FH + (c2 + 1) * FC]),
                                    start=True,
                                    stop=True,
                                )
                            pt = prpool.tile([P, FH], f32r, tag="pt")
                            nc.scalar.activation(
                                pt[:], sc[:], AF.Exp,
                                bias=mb[:, tt:tt + 1], scale=0.125,
                            )
                            for c2 in range(nck):
                                nc.tensor.matmul(
                                    av[c2][:],
                                    lhsT=_r(V[:, tt, h * VW:(h + 1) * VW]),
                                    rhs=_r(pt[:, c2 * FC:(c2 + 1) * FC]),
                                    start=(tt == 0),
                                    stop=(tt == nft - 1),
                                )
                        dn = nrm.tile([1, FH], f32, tag="dn")
                        for c2 in range(nck):
                            nc.vector.tensor_copy(
                                dn[:, c2 * FC:(c2 + 1) * FC],
                                av[c2][DH:DH + 1, :],
                            )
                        rc = nrm.tile([1, FH], f32, tag="rc")
                        nc.vector.reciprocal(rc[:], dn[:])
                        rb = nrm.tile([DH, FH], f32, tag="rb")
                        nc.gpsimd.partition_broadcast(rb[:], rc[:])
                        on = outp.tile([DH, FH], f32, tag="on")
                        for c2 in range(nck):
                            nc.vector.tensor_tensor(
                                on[:, c2 * FC:(c2 + 1) * FC],
                                av[c2][0:DH, :],
                                rb[:, c2 * FC:(c2 + 1) * FC],
                                op=mybir.AluOpType.mult,
                            )
                        nc.sync.dma_start(
                            outT_d[h * DH:(h + 1) * DH, fh * FH:(fh + 1) * FH],
                            on[:],
                        )

    nc.compile()
    return nc


_PROGRAM = None
LAST_RESULT = None


def _program():
    global _PROGRAM
    if _PROGRAM is None:
        _PROGRAM = _build_program()
    return _PROGRAM


def _in_maps(from_tensor, to_tensor, to_mask, Wq, bq, Wk, bk, Wv, bv):
    maps = []
    for core in range(8):
        b, g = core // 2, core % 2
        cols = slice(g * DL, (g + 1) * DL)
        adder = ((1.0 - to_mask[b].astype(np.float32)) * -10000.0)
        maps.append({
            "x_from": np.ascontiguousarray(from_tensor[b]),
            "x_to": np.ascontiguousarray(to_tensor[b]),
            "wq": np.ascontiguousarray(Wq[:, cols]),
            "wk": np.ascontiguousarray(Wk[:, cols]),
            "wv": np.ascontiguousarray(Wv[:, cols]),
            "bq": np.ascontiguousarray(bq[cols]).reshape(1, DL),
            "bk": np.ascontiguousarray(bk[cols]).reshape(1, DL),
            "bv": np.ascontiguousarray(bv[cols]).reshape(1, DL),
            "mask_bias": np.ascontiguousarray(
                adder.reshape(NFT, P).T
            ),
        })
    return maps


def kernel(from_tensor, to_tensor, from_mask, to_mask, Wq, bq, Wk, bk, Wv, bv,
           **run_kwargs):
    from_tensor = np.asarray(from_tensor, dtype=np.float32)
    to_tensor = np.asarray(to_tensor, dtype=np.float32)
    to_mask = np.asarray(to_mask)
    Wq, Wk, Wv = (np.asarray(w, dtype=np.float32) for w in (Wq, Wk, Wv))
    bq, bk, bv = (np.asarray(v, dtype=np.float32) for v in (bq, bk, bv))

    nc = _program()
    maps = _in_maps(from_tensor, to_tensor, to_mask, Wq, bq, Wk, bk, Wv, bv)
    res = run_bass_kernel_spmd(nc, maps, list(range(8)), **run_kwargs)
    global LAST_RESULT
    LAST_RESULT = res

    out = np.empty((B, S, DM), dtype=np.float32)
    for core in range(8):
        b, g = core // 2, core % 2
        out[b, :, g * DL:(g + 1) * DL] = res.results[core]["outT"].T
    return out


if __name__ == "__main__":
    rng = np.random.default_rng(0)
    ins = {
        "from_tensor": rng.standard_normal((B, S, DM), dtype=np.float32),
        "to_tensor": rng.standard_normal((B, S, DM), dtype=np.float32),
        "from_mask": np.ones((B, S), dtype=np.int32),
        "to_mask": np.ones((B, S), dtype=np.int32),
        "Wq": (rng.standard_normal((DM, DM), dtype=np.float32) * 0.02),
        "bq": np.zeros(DM, dtype=np.float32),
        "Wk": (rng.standard_normal((DM, DM), dtype=np.float32) * 0.02),
        "bk": np.zeros(DM, dtype=np.float32),
        "Wv": (rng.standard_normal((DM, DM), dtype=np.float32) * 0.02),
        "bv": np.zeros(DM, dtype=np.float32),
    }
    out = kernel(**ins)
    print(out.shape, out.dtype, np.abs(out).max())


# revision 11
# speedup vs baseline: 1.1490x; 1.1490x over previous
"""Bass/Trainium2 kernel for nn_AttentionLayer (B=4, S=2048, H=16, DH=64).

Sharding: 8 cores = 4 batches x 2 head-groups (8 heads each). Each core
computes its batch's full S x S attention for its 8 heads; no cross-core
communication. Host slices inputs per core and transposes/concats outputs.

Per-core dataflow (all fp32 storage; matmuls run as float32r bitcasts):
  x_to -> PE-transpose -> toT [m, t] -> K^T [d, t] and V [t, d|1]
  x_from -> PE-transpose -> fromT [m, f] -> Q^T [d, f]
  per head h, per f-half:
    scores^T[t, f] = K_h Q_h^T       (PE, K=64 row-group per head parity)
    probs^T = exp(0.125*scores^T + mask_bias)   (scalar engine, PSUM->SBUF)
    out^T[d|sum, f] += [V_h|1]^T probs^T        (PE, M=65, PSUM-accumulated)
    out = out^T[0:64] * (1/out^T[64])           (DVE + gpsimd broadcast)
Output per core: outT [512, 2048] (head-major rows); host takes outT.T.
"""

import sys

sys.path.insert(0, "/opt/trn_rl_repo")

import ml_dtypes
import numpy as np

import concourse.bass as bass
import concourse.tile as tile
from concourse import bacc, mybir
from concourse.bass_utils import run_bass_kernel_spmd
from concourse.masks import make_identity

B, S, H, DH = 4, 2048, 16, 64
DM = H * DH          # 1024 model dim
HL = 8               # heads per core
DL = HL * DH         # 512 projected dim per core
P = 128
NFT = S // P         # 16 sequence tiles
NMT = DM // P        # 8 model-dim tiles
NDT = DL // P        # 4 projected-dim tiles
FC = 512             # matmul free-dim chunk
VW = DH + 1          # V columns per head incl. ones column

f32 = mybir.dt.float32
f32r = mybir.dt.float32r
bf16 = mybir.dt.bfloat16
AF = mybir.ActivationFunctionType


def _r(ap):
    return ap


def _build_program(s=S):
    nc = bacc.Bacc("TRN2", target_bir_lowering=False, num_devices=8)
    nft = s // P

    x_from_d = nc.dram_tensor("x_from", [s, DM], bf16, kind="ExternalInput")
    x_to_d = nc.dram_tensor("x_to", [s, DM], bf16, kind="ExternalInput")
    wq_d = nc.dram_tensor("wq", [DM, DL], bf16, kind="ExternalInput")
    wk_d = nc.dram_tensor("wk", [DM, DL], bf16, kind="ExternalInput")
    wv_d = nc.dram_tensor("wv", [DM, DL], bf16, kind="ExternalInput")
    bq_d = nc.dram_tensor("bq", [1, DL], bf16, kind="ExternalInput")
    bk_d = nc.dram_tensor("bk", [1, DL], bf16, kind="ExternalInput")
    bv_d = nc.dram_tensor("bv", [1, DL], bf16, kind="ExternalInput")
    mb_d = nc.dram_tensor("mask_bias", [P, nft], f32, kind="ExternalInput")
    outT_d = nc.dram_tensor("outT", [DL, s], f32, kind="ExternalOutput")

    xf_t = x_from_d.rearrange("(ft p) m -> ft p m", p=P)
    xt_t = x_to_d.rearrange("(ft p) m -> ft p m", p=P)

    with tile.TileContext(nc) as tc:
        with tc.tile_pool(name="const", bufs=1) as const, \
             tc.tile_pool(name="big", bufs=1) as big:
            ident_f = const.tile([P, P], f32)
            make_identity(nc, ident_f[:])
            ident = const.tile([P, P], bf16)
            nc.vector.tensor_copy(ident[:], ident_f[:])
            ones_f = const.tile([P, FC], f32)
            nc.gpsimd.memset(ones_f[:], 1.0)
            ones_row = const.tile([1, FC], bf16)
            nc.vector.tensor_copy(ones_row[:], ones_f[0:1, :])
            mb = const.tile([P, nft], f32)
            nc.sync.dma_start(mb[:], mb_d[:])
            bq_sb = const.tile([1, DL], bf16)
            nc.sync.dma_start(bq_sb[:], bq_d[:])
            bk_sb = const.tile([1, DL], bf16)
            nc.sync.dma_start(bk_sb[:], bk_d[:])
            bv_sb = const.tile([1, DL], bf16)
            nc.sync.dma_start(bv_sb[:], bv_d[:])

            QT = big.tile([P, NDT, s], bf16)   # Q^T: [d%128, d//128, f]
            KT = big.tile([P, NDT, s], bf16)   # K^T: [d%128, d//128, t]
            V = big.tile([P, nft, HL * VW], bf16)  # [t%128, t//128, h*65+j]
            nc.vector.tensor_copy(
                V.rearrange("p t (h d) -> p t h d", d=VW)[:, :, :, DH],
                ones_f[:, 0:nft * HL].rearrange("p (t h) -> p t h", h=HL),
            )

            def transpose_in(x_tiled, dst, xpool, tps):
                # dst[m%128, m//128, s] = x[s, m]
                for ft in range(nft):
                    xt = xpool.tile([P, DM], bf16, tag="x")
                    nc.sync.dma_start(xt[:], x_tiled[ft])
                    ps = tps.tile([P, DM], bf16, tag="tp")
                    for mt in range(NMT):
                        sl = slice(mt * P, (mt + 1) * P)
                        nc.tensor.transpose(
                            _r(ps[:, sl]), _r(xt[:, sl]), _r(ident[:])
                        )
                    nc.vector.tensor_copy(
                        dst[:, :, ft * P:(ft + 1) * P],
                        ps.rearrange("p (mt c) -> p mt c", mt=NMT),
                    )

            def project_T(w_dram, b_sb, xT, dst, wpool, pps):
                # dst[d%128, d//128, s] = sum_m w[m, d] * xT[m, s] + b[d]
                for dt in range(NDT):
                    wt = wpool.tile([P, NMT, P], bf16, tag="w")
                    nc.sync.dma_start(
                        wt[:],
                        w_dram[:, dt * P:(dt + 1) * P].rearrange(
                            "(mt p) d -> p mt d", p=P
                        ),
                    )
                    for c in range(s // FC):
                        ps = pps.tile([P, FC], f32, tag="pj")
                        for mt in range(NMT):
                            nc.tensor.matmul(
                                ps[:],
                                lhsT=_r(wt[:, mt, :]),
                                rhs=_r(xT[:, mt, c * FC:(c + 1) * FC]),
                                start=(mt == 0),
                                stop=False,
                            )
                        nc.tensor.matmul(
                            ps[:],
                            lhsT=_r(b_sb[0:1, dt * P:(dt + 1) * P]),
                            rhs=_r(ones_row[0:1, :]),
                            start=False,
                            stop=True,
                        )
                        nc.vector.tensor_copy(
                            dst[:, dt, c * FC:(c + 1) * FC], ps[:]
                        )

            with tc.tile_pool(name="xload", bufs=2) as xpool, \
                 tc.tile_pool(name="wpool", bufs=2) as wpool, \
                 tc.tile_pool(name="tp_ps", bufs=2, space="PSUM") as tps, \
                 tc.tile_pool(name="pj_ps", bufs=2, space="PSUM") as pps:

                with tc.tile_pool(name="toT_pool", bufs=1) as toT_pool:
                    toT = toT_pool.tile([P, NMT, s], bf16)
                    transpose_in(xt_t, toT, xpool, tps)
                    project_T(wk_d, bk_sb, toT, KT, wpool, pps)
                    # V[t, d] = sum_m toT[m, t] * wv[m, d] + bv[d]
                    wv_sb = toT_pool.tile([P, NMT, DL], bf16)
                    nc.sync.dma_start(
                        wv_sb[:], wv_d.rearrange("(mt p) d -> p mt d", p=P)
                    )
                    for tt in range(nft):
                        ps = pps.tile([P, FC], f32, tag="pj")
                        for mt in range(NMT):
                            nc.tensor.matmul(
                                ps[:],
                                lhsT=_r(toT[:, mt, tt * P:(tt + 1) * P]),
                                rhs=_r(wv_sb[:, mt, :]),
                                start=(mt == 0),
                                stop=False,
                            )
                        nc.tensor.matmul(
                            ps[:],
                            lhsT=_r(ones_row[0:1, 0:P]),
                            rhs=_r(bv_sb[:]),
                            start=False,
                            stop=True,
                        )
                        nc.vector.tensor_copy(
                            V.rearrange("p t (h d) -> p t h d", d=VW)[
                                :, tt, :, 0:DH
                            ],
                            ps.rearrange("p (h d) -> p h d", d=DH),
                        )

                with tc.tile_pool(name="fromT_pool", bufs=1) as fromT_pool:
                    fromT = fromT_pool.tile([P, NMT, s], bf16)
                    transpose_in(xf_t, fromT, xpool, tps)
                    project_T(wq_d, bq_sb, fromT, QT, wpool, pps)

            # ---- attention ----
            with tc.tile_pool(name="sc_ps", bufs=2, space="PSUM") as scps, \
                 tc.tile_pool(name="av_ps", bufs=2, space="PSUM") as avps, \
                 tc.tile_pool(name="probs", bufs=3) as prpool, \
                 tc.tile_pool(name="norm", bufs=2) as nrm, \
                 tc.tile_pool(name="outp", bufs=2) as outp:
                FH = min(s, 2 * FC)  # f-half width
                nfh = s // FH
                nck = FH // FC
                for h in range(HL):
                    pr, ro = h // 2, (h % 2) * 64
                    for fh in range(nfh):
                        av = [
                            avps.tile([VW, FC], f32, tag=f"av{i}",
                                      name=f"av{i}")
                            for i in range(nck)
                        ]
                        for tt in range(nft):
                            sc = scps.tile([P, FH], f32, tag="sc")
                            for c2 in range(nck):
                                nc.tensor.matmul(
                                    sc[:, c2 * FC:(c2 + 1) * FC],
                                    lhsT=_r(KT[ro:ro + 64, pr, tt * P:(tt + 1) * P]),
                                    rhs=_r(QT[ro:ro + 64, pr,
                                              fh * FH + c2 * FC:
                                              fh * FH + (c2 + 1) * FC]),
                                    start=True,
                                    stop=True,
                                )
                            pt = prpool.tile([P, FH], bf16, tag="pt")
                            nc.scalar.activation(
                                pt[:], sc[:], AF.Exp,
                                bias=mb[:, tt:tt + 1], scale=0.125,
                            )
                            for c2 in range(nck):
                                nc.tensor.matmul(
                                    av[c2][:],
                                    lhsT=_r(V[:, tt, h * VW:(h + 1) * VW]),
                                    rhs=_r(pt[:, c2 * FC:(c2 + 1) * FC]),
                                    start=(tt == 0),
                                    stop=(tt == nft - 1),
                                )
                        dn = nrm.tile([1, FH], f32, tag="dn")
                        for c2 in range(nck):
                            nc.vector.tensor_copy(
                                dn[:, c2 * FC:(c2 + 1) * FC],
                                av[c2][DH:DH + 1, :],
                            )
                        rc = nrm.tile([1, FH], f32, tag="rc")
                        nc.vector.reciprocal_approx_fast(rc[:], dn[:])
                        rb = nrm.tile([DH, FH], f32, tag="rb")
                        nc.gpsimd.partition_broadcast(rb[:], rc[:])
                        on = outp.tile([DH, FH], f32, tag="on")
                        for c2 in range(nck):
                            nc.vector.tensor_tensor(
                                on[:, c2 * FC:(c2 + 1) * FC],
                                av[c2][0:DH, :],
                                rb[:, c2 * FC:(c2 + 1) * FC],
                                op=mybir.AluOpType.mult,
                            )
                        nc.sync.dma_start(
                            outT_d[h * DH:(h + 1) * DH, fh * FH:(fh + 1) * FH],
                            on[:],
                        )

    nc.compile()
    return nc


_PROGRAM = None
LAST_RESULT = None


def _program():
    global _PROGRAM
    if _PROGRAM is None:
        _PROGRAM = _build_program()
    return _PROGRAM


def _in_maps(from_tensor, to_tensor, to_mask, Wq, bq, Wk, bk, Wv, bv):
    maps = []
    for core in range(8):
        b, g = core // 2, core % 2
        cols = slice(g * DL, (g + 1) * DL)
        adder = ((1.0 - to_mask[b].astype(np.float32)) * -10000.0)
        maps.append({
            "x_from": from_tensor[b].astype(ml_dtypes.bfloat16),
            "x_to": to_tensor[b].astype(ml_dtypes.bfloat16),
            "wq": Wq[:, cols].astype(ml_dtypes.bfloat16),
            "wk": Wk[:, cols].astype(ml_dtypes.bfloat16),
            "wv": Wv[:, cols].astype(ml_dtypes.bfloat16),
            "bq": bq[cols].reshape(1, DL).astype(ml_dtypes.bfloat16),
            "bk": bk[cols].reshape(1, DL).astype(ml_dtypes.bfloat16),
            "bv": bv[cols].reshape(1, DL).astype(ml_dtypes.bfloat16),
            "mask_bias": np.ascontiguousarray(
                adder.reshape(NFT, P).T
            ),
        })
    return maps


def kernel(from_tensor, to_tensor, from_mask, to_mask, Wq, bq, Wk, bk, Wv, bv,
           **run_kwargs):
    from_tensor = np.asarray(from_tensor, dtype=np.float32)
    to_tensor = np.asarray(to_tensor, dtype=np.float32)
    to_mask = np.asarray(to_mask)
    Wq, Wk, Wv = (np.asarray(w, dtype=np.float32) for w in (Wq, Wk, Wv))
    bq, bk, bv = (np.asarray(v, dtype=np.float32) for v in (bq, bk, bv))

    nc = _program()
    maps = _in_maps(from_tensor, to_tensor, to_mask, Wq, bq, Wk, bk, Wv, bv)
    res = run_bass_kernel_spmd(nc, maps, list(range(8)), **run_kwargs)
    global LAST_RESULT
    LAST_RESULT = res

    out = np.empty((B, S, DM), dtype=np.float32)
    for core in range(8):
        b, g = core // 2, core % 2
        out[b, :, g * DL:(g + 1) * DL] = res.results[core]["outT"].T
    return out


if __name__ == "__main__":
    rng = np.random.default_rng(0)
    ins = {
        "from_tensor": rng.standard_normal((B, S, DM), dtype=np.float32),
        "to_tensor": rng.standard_normal((B, S, DM), dtype=np.float32),
        "from_mask": np.ones((B, S), dtype=np.int32),
        "to_mask": np.ones((B, S), dtype=np.int32),
        "Wq": (rng.standard_normal((DM, DM), dtype=np.float32) * 0.02),
        "bq": np.zeros(DM, dtype=np.float32),
        "Wk": (rng.standard_normal((DM, DM), dtype=np.float32) * 0.02),
        "bk": np.zeros(DM, dtype=np.float32),
        "Wv": (rng.standard_normal((DM, DM), dtype=np.float32) * 0.02),
        "bv": np.zeros(DM, dtype=np.float32),
    }
    out = kernel(**ins)
    print(out.shape, out.dtype, np.abs(out).max())
